# revision 9
# baseline (speedup 1.0000x reference)
"""2-layer GCN + linear classifier on 8 trn2 NeuronCores.

out = relu(A @ relu(A @ x @ W1 + b1) @ W2 + b2) @ Wc + bc
with A = D^-1/2 (adj + I) D^-1/2, N=50000 nodes, E=800000 edges, d=64.

Strategy (dst-partitioned graph parallel):
 - Host routes edges by dst shard (6250 nodes/core), sorts per 128-node dst
   block, splits by src < 32768 (dma_gather int16 index limit), pads each
   (block, range) group to whole 128-edge batches uniformly across cores
   (same SPMD program on every core), and precomputes per-edge norm =
   rsqrt(deg[src]) * rsqrt(deg[dst]) plus in-block dst offsets.
 - Device: per 128-edge batch, dma_gather pulls feature rows (x for layer 1,
   t2 = h2 @ W2 for layer 2); DVE builds a norm-scaled one-hot selection
   matrix M[e, d] = norm[e] * (dstloc[e] == d) in one tensor_scalar op; PE
   computes gathered.T @ M accumulating aggT [64f, 128n] per dst block in
   PSUM. Layer-1 epilogue per block: aggT -> W1 matmul -> relu+b1 ->
   t2 = h2 @ W2 -> DRAM slice. AllGather exchanges t2 slices between layers.
   Layer-2 epilogue: relu+b2 -> classifier matmul (@Wc) -> +bc -> out slice.
"""

import hashlib
import math
import os

import numpy as np

N = 50000
E = 800000
D = 64
NCLS = 16
NCORES = 8
NPC = N // NCORES            # 6250 nodes per core
P = 128
NBLK = math.ceil(NPC / P)    # 49 dst blocks per core (last has 106 nodes)
LO = 32768                   # int16 index range split
G = 8                        # dst blocks per PSUM group

_cache: dict = {}


def _preprocess(edge_index: np.ndarray):
    """Route/sort/pad edges; build per-core device arrays."""
    src = np.concatenate([edge_index[0], np.arange(N, dtype=np.int64)])
    dst = np.concatenate([edge_index[1], np.arange(N, dtype=np.int64)])
    deg = np.bincount(dst, minlength=N).astype(np.float64)
    inv = 1.0 / np.sqrt(np.maximum(deg, 1.0))
    inv[deg == 0] = 0.0
    norm = (inv[src] * inv[dst]).astype(np.float32)

    core = dst // NPC
    ldst = dst - core * NPC
    blk = ldst // P
    dloc = (ldst % P).astype(np.float32)
    rng = (src >= LO).astype(np.int64)

    key = (core * NBLK + blk) * 2 + rng
    order = np.argsort(key, kind="stable")
    src, norm, dloc, key = src[order], norm[order], dloc[order], key[order]
    rng = rng[order]

    counts = np.bincount(key, minlength=NCORES * NBLK * 2).reshape(
        NCORES, NBLK, 2)
    nb = np.ceil(counts / P).astype(np.int64).max(axis=0)  # [NBLK, 2]

    # batch-column layout: for each group: [lo batches of its blocks][hi ...]
    groups = [list(range(g * G, min((g + 1) * G, NBLK)))
              for g in range(math.ceil(NBLK / G))]
    col0 = np.zeros((NBLK, 2), np.int64)
    run_span = []          # [(lo_start, lo_end), (hi_start, hi_end)] per group
    c = 0
    for blocks in groups:
        spans = []
        for r in (0, 1):
            s = c
            for b in blocks:
                col0[b, r] = c
                c += nb[b, r]
            spans.append((s, c))
        run_span.append(spans)
    totb = c

    # flat position of every edge within its core's [totb*128] stream
    starts = np.zeros((NCORES, NBLK, 2), np.int64)
    flat = counts.reshape(-1)
    np.cumsum(flat[:-1], out=starts.reshape(-1)[1:])
    pos = np.empty(src.shape[0], np.int64)
    for ci in range(NCORES):
        for b in range(NBLK):
            for r in (0, 1):
                off = starts[ci, b, r]
                cnt = counts[ci, b, r]
                pos[off:off + cnt] = col0[b, r] * P + np.arange(cnt)

    idx_flat = np.zeros((NCORES, totb * P), np.int16)
    dloc_flat = np.zeros((NCORES, totb * P), np.float32)
    norm_flat = np.zeros((NCORES, totb * P), np.float32)
    core_off = np.concatenate([[0], np.cumsum(counts.sum(axis=(1, 2)))])
    sadj = (src - rng * LO).astype(np.int16)
    for ci in range(NCORES):
        sl = slice(core_off[ci], core_off[ci + 1])
        idx_flat[ci, pos[sl]] = sadj[sl]
        dloc_flat[ci, pos[sl]] = dloc[sl]
        norm_flat[ci, pos[sl]] = norm[sl]

    # dma_gather index layout: [128, n/16] int16, idx i at [i%16, i//16],
    # replicated across the 8 gpsimd cores' 16-partition stripes
    idx_w = idx_flat.reshape(NCORES, totb * P // 16, 16).transpose(0, 2, 1)
    idx_w = np.ascontiguousarray(np.tile(idx_w, (1, 8, 1)))       # [8,128,n/16]
    dloc_c = np.ascontiguousarray(
        dloc_flat.reshape(NCORES, totb, P).transpose(0, 2, 1))    # [8,128,totb]
    norm_c = np.ascontiguousarray(
        norm_flat.reshape(NCORES, totb, P).transpose(0, 2, 1))

    return {
        "nb": nb, "groups": groups, "col0": col0, "run_span": run_span,
        "totb": totb, "idx_w": idx_w, "dloc_c": dloc_c, "norm_c": norm_c,
    }


def _build(pp):
    import concourse.mybir as mybir
    import concourse.tile as tile
    from concourse import bacc
    from concourse.library_config import mlp

    stage = os.environ.get("GCN_STAGE", "full")  # l1 | nocc | full
    nb, groups, col0, run_span, totb = (
        pp["nb"], pp["groups"], pp["col0"], pp["run_span"], pp["totb"])

    nc = bacc.Bacc("TRN2", target_bir_lowering=False, debug=False,
                   num_devices=NCORES)
    f32 = mybir.dt.float32

    x_d = nc.dram_tensor("x", [N, D], f32, kind="ExternalInput")
    idx_d = nc.dram_tensor("idxs", [P, totb * 8], mybir.dt.int16,
                           kind="ExternalInput")
    dl_d = nc.dram_tensor("dstloc", [P, totb], f32, kind="ExternalInput")
    nm_d = nc.dram_tensor("normv", [P, totb], f32, kind="ExternalInput")
    io_d = nc.dram_tensor("iota_in", [P, P], f32, kind="ExternalInput")
    w1_d = nc.dram_tensor("w1", [D, D], f32, kind="ExternalInput")
    w2_d = nc.dram_tensor("w2", [D, D], f32, kind="ExternalInput")
    wc_d = nc.dram_tensor("wc", [D, NCLS], f32, kind="ExternalInput")
    b1_d = nc.dram_tensor("b1", [D, 1], f32, kind="ExternalInput")
    b2_d = nc.dram_tensor("b2", [D, 1], f32, kind="ExternalInput")
    bc_d = nc.dram_tensor("bc_rep", [P, NCLS], f32, kind="ExternalInput")
    out_d = nc.dram_tensor("out", [NPC, NCLS], f32, kind="ExternalOutput")

    relu = mybir.ActivationFunctionType.Relu
    is_eq = mybir.AluOpType.is_equal
    mult = mybir.AluOpType.mult

    with tile.TileContext(nc) as tc:
        with (
            tc.tile_pool(name="const", bufs=1) as const,
            tc.tile_pool(name="gpool", bufs=3) as gpool,
            tc.tile_pool(name="mpool", bufs=8) as mpool,
            tc.tile_pool(name="epil", bufs=4) as epil,
            tc.tile_pool(name="pacc", bufs=2, space="PSUM") as pacc,
            tc.tile_pool(name="pepi", bufs=3, space="PSUM") as pepi,
            tc.tile_pool(name="dram", bufs=1, space="DRAM") as dram,
        ):
            nc.gpsimd.load_library(mlp)

            idx_sb = const.tile([P, totb * 8], mybir.dt.int16)
            nc.sync.dma_start(idx_sb[:], idx_d[:])
            dl_sb = const.tile([P, totb], f32)
            nc.sync.dma_start(dl_sb[:], dl_d[:])
            nm_sb = const.tile([P, totb], f32)
            nc.sync.dma_start(nm_sb[:], nm_d[:])
            io_sb = const.tile([P, P], f32)
            nc.sync.dma_start(io_sb[:], io_d[:])
            w1_sb = const.tile([D, D], f32)
            nc.sync.dma_start(w1_sb[:], w1_d[:])
            w2_sb = const.tile([D, D], f32)
            nc.sync.dma_start(w2_sb[:], w2_d[:])
            wc_sb = const.tile([D, NCLS], f32)
            nc.sync.dma_start(wc_sb[:], wc_d[:])
            b1_sb = const.tile([D, 1], f32)
            nc.sync.dma_start(b1_sb[:], b1_d[:])
            b2_sb = const.tile([D, 1], f32)
            nc.sync.dma_start(b2_sb[:], b2_d[:])
            bc_sb = const.tile([P, NCLS], f32)
            nc.sync.dma_start(bc_sb[:], bc_d[:])

            t2_slice = nc.dram_tensor("t2_slice", [NPC, D], f32)
            t2_full = nc.dram_tensor("t2_full", [N, D], f32,
                                     addr_space="Shared")

            def layer(table_lo, table_hi, is_last):
                for gi, blocks in enumerate(groups):
                    acc = pacc.tile([D, G * P], f32, tag="acc")
                    gts = {}
                    for r, tbl in ((0, table_lo), (1, table_hi)):
                        c0, c1 = run_span[gi][r]
                        if c1 > c0:
                            n_idx = (c1 - c0) * P
                            gt = gpool.tile([P, c1 - c0, D], f32, tag="g")
                            nc.gpsimd.dma_gather(
                                gt[:], tbl, idx_sb[:, c0 * 8:c1 * 8],
                                n_idx, n_idx, D, single_packet=False)
                            gts[r] = (gt, c0)
                    for bi, b in enumerate(blocks):
                        pairs = [(r, j) for r in (0, 1)
                                 for j in range(nb[b, r])]
                        aslc = acc[:, bi * P:(bi + 1) * P]
                        for k, (r, j) in enumerate(pairs):
                            col = col0[b, r] + j
                            gt, c0 = gts[r]
                            m = mpool.tile([P, P], f32, tag="m")
                            nc.vector.tensor_scalar(
                                out=m[:], in0=io_sb[:],
                                scalar1=dl_sb[:, col:col + 1],
                                scalar2=nm_sb[:, col:col + 1],
                                op0=is_eq, op1=mult)
                            nc.tensor.matmul(
                                aslc, gt[:, col - c0, :], m[:],
                                start=(k == 0), stop=(k == len(pairs) - 1))
                        rows = min(P, NPC - b * P)
                        if not is_last:
                            ag = epil.tile([D, P], f32, tag="ag")
                            nc.vector.tensor_copy(ag[:], aslc)
                            hp = pepi.tile([D, P], f32, tag="epi")
                            nc.tensor.matmul(hp[:], w1_sb[:], ag[:],
                                             start=True, stop=True)
                            h2 = epil.tile([D, P], f32, tag="h2")
                            nc.scalar.activation(h2[:], hp[:], relu,
                                                 bias=b1_sb[:, :1])
                            tp = pepi.tile([P, D], f32, tag="epi")
                            nc.tensor.matmul(tp[:], h2[:], w2_sb[:],
                                             start=True, stop=True)
                            ts = epil.tile([P, D], f32, tag="ts")
                            nc.vector.tensor_copy(ts[:], tp[:])
                            nc.sync.dma_start(
                                t2_slice[b * P:b * P + rows, :], ts[:rows, :])
                            if stage == "l1":
                                nc.sync.dma_start(
                                    out_d[b * P:b * P + rows, :],
                                    ts[:rows, :NCLS])
                        else:
                            h3 = epil.tile([D, P], f32, tag="h2")
                            nc.scalar.activation(h3[:], aslc, relu,
                                                 bias=b2_sb[:, :1])
                            op = pepi.tile([P, NCLS], f32, tag="epi")
                            nc.tensor.matmul(op[:], h3[:], wc_sb[:],
                                             start=True, stop=True)
                            os_ = epil.tile([P, NCLS], f32, tag="ts")
                            nc.vector.tensor_add(os_[:], op[:], bc_sb[:])
                            nc.sync.dma_start(
                                out_d[b * P:b * P + rows, :], os_[:rows, :])

            layer(x_d[0:LO, :], x_d[LO:N, :], is_last=False)

            if stage != "l1":
                if stage != "nocc":
                    nc.gpsimd.collective_compute(
                        "AllGather", mybir.AluOpType.bypass,
                        replica_groups=[list(range(NCORES))],
                        ins=[t2_slice[:]], outs=[t2_full[:]])
                layer(t2_full[0:LO, :], t2_full[LO:N, :], is_last=True)

    nc.compile()
    return nc


def _get(edge_index: np.ndarray):
    h = (hashlib.sha1(np.ascontiguousarray(edge_index)).hexdigest()
         + os.environ.get("GCN_STAGE", "full"))
    if h not in _cache:
        pp = _preprocess(np.asarray(edge_index, dtype=np.int64))
        nc = _build(pp)
        _cache.clear()
        _cache[h] = (pp, nc)
    return _cache[h]


def kernel(x, edge_index, W1, b1, W2, b2, Wc, bc):
    from concourse.bass_utils import run_bass_kernel_spmd

    pp, nc = _get(np.asarray(edge_index))
    x = np.ascontiguousarray(np.asarray(x, np.float32))
    iota = np.tile(np.arange(P, dtype=np.float32), (P, 1))
    common = {
        "x": x,
        "iota_in": iota,
        "w1": np.ascontiguousarray(np.asarray(W1, np.float32)),
        "w2": np.ascontiguousarray(np.asarray(W2, np.float32)),
        "wc": np.ascontiguousarray(np.asarray(Wc, np.float32)),
        "b1": np.asarray(b1, np.float32).reshape(D, 1),
        "b2": np.asarray(b2, np.float32).reshape(D, 1),
        "bc_rep": np.tile(np.asarray(bc, np.float32).reshape(1, NCLS),
                          (P, 1)),
    }
    in_maps = [dict(common,
                    idxs=pp["idx_w"][c],
                    dstloc=pp["dloc_c"][c],
                    normv=pp["norm_c"][c]) for c in range(NCORES)]

    trace = bool(int(os.environ.get("GCN_TRACE", "0")))
    kw = {}
    if trace:
        kw["tmpdir"] = os.environ.get("GCN_TRACE_DIR") or None
        tc_env = os.environ.get("GCN_TRACE_CORES", "0")
        kw["trace_cores"] = [int(c) for c in tc_env.split(",")]
    res = run_bass_kernel_spmd(
        nc, in_maps, core_ids=list(range(NCORES)), trace=trace, **kw)
    if res.exec_time_ns is not None:
        print(f"HW exec time: {res.exec_time_ns} ns")
        if res.instructions_and_trace is not None:
            print(f"trace: {res.instructions_and_trace[1]}")
    out = np.concatenate([res.results[c]["out"] for c in range(NCORES)],
                         axis=0)
    return out.astype(np.float32)


# revision 22
# speedup vs baseline: 2.2853x; 2.2853x over previous
"""2-layer GCN + linear classifier on 8 trn2 NeuronCores.

out = relu(A @ relu(A @ x @ W1 + b1) @ W2 + b2) @ Wc + bc
with A = D^-1/2 (adj + I) D^-1/2, N=50000 nodes, E=800000 edges, d=64.

Strategy (dst-partitioned graph parallel):
 - Host routes edges by dst shard (6250 nodes/core), sorts per 128-node dst
   block, splits by src < 32768 (dma_gather int16 index limit), pads each
   (block, range) group to whole 128-edge batches uniformly across cores
   (same SPMD program everywhere), and precomputes per-edge norm =
   rsqrt(deg[src]) * rsqrt(deg[dst]) plus in-block dst offsets.
 - Device, per gather chunk (<=64 batches): dma_gather split across 4 SWDGE
   queues pulls the source feature rows (x for layer 1, t2 = h2 @ W2 for
   layer 2); one bulk DVE tensor_tensor builds the one-hot selection
   matrices M[e, d] = (dstloc[e] == d) in bf16; one bulk tensor_tensor
   scales messages by norm and casts to bf16. PE contracts msg.T @ M per
   128-edge batch, accumulating aggT [64f, 128n] per dst block in PSUM.
   Layer-1 epilogue per block: aggT -> @W1 -> relu+b1 -> @W2 -> t2 slice
   in DRAM. One AllGather exchanges t2 slices between layers. Layer-2
   epilogue: relu+b2 -> @Wc -> +bc -> out slice.
"""

import hashlib
import math
import os

import numpy as np

N = 50000
E = 800000
D = 64
NCLS = 16
NCORES = 8
NPC = N // NCORES            # 6250 nodes per core
P = 128
NBLK = math.ceil(NPC / P)    # 49 dst blocks per core (last has 106 nodes)
LO = 32768                   # int16 index range split
G = 4                        # dst blocks per PSUM group (1 PSUM bank each)
CHUNK = 48                   # max batches per gather chunk
NQ = int(os.environ.get("GCN_NQ", "4"))      # swdge queues

_cache: dict = {}


def _preprocess(edge_index: np.ndarray):
    """Route/sort/pad edges; build per-core device arrays."""
    import ml_dtypes

    src = np.concatenate([edge_index[0], np.arange(N, dtype=np.int64)])
    dst = np.concatenate([edge_index[1], np.arange(N, dtype=np.int64)])
    deg = np.bincount(dst, minlength=N).astype(np.float64)
    inv = 1.0 / np.sqrt(np.maximum(deg, 1.0))
    inv[deg == 0] = 0.0
    norm = (inv[src] * inv[dst]).astype(np.float32)

    core = dst // NPC
    ldst = dst - core * NPC
    blk = ldst // P
    dloc = (ldst % P).astype(np.float32)
    rng = (src >= LO).astype(np.int64)

    key = (core * NBLK + blk) * 2 + rng
    order = np.argsort(key, kind="stable")
    src, norm, dloc, rng = src[order], norm[order], dloc[order], rng[order]

    counts = np.bincount(key[order], minlength=NCORES * NBLK * 2).reshape(
        NCORES, NBLK, 2)
    nb = np.ceil(counts / P).astype(np.int64).max(axis=0)      # [NBLK, 2]

    groups = [list(range(g * G, min((g + 1) * G, NBLK)))
              for g in range(math.ceil(NBLK / G))]
    col0 = np.zeros((NBLK, 2), np.int64)
    chunks = []          # (c0, c1, range) batch-col spans
    grp_chunks = []      # per group: list of chunk ids
    c = 0
    for blocks in groups:
        ids = []
        for r in (0, 1):
            s = c
            for b in blocks:
                col0[b, r] = c
                c += nb[b, r]
            e = c
            cc = s
            while cc < e:
                ids.append(len(chunks))
                chunks.append((cc, min(cc + CHUNK, e), r))
                cc = min(cc + CHUNK, e)
        grp_chunks.append(ids)
    totb = c

    colblk = np.zeros(totb, np.int64)          # batch col -> dst block
    for b in range(NBLK):
        for r in (0, 1):
            colblk[col0[b, r]:col0[b, r] + nb[b, r]] = b

    # flat position of every edge within its core's [totb*128] stream
    starts = np.zeros((NCORES, NBLK, 2), np.int64)
    flat = counts.reshape(-1)
    np.cumsum(flat[:-1], out=starts.reshape(-1)[1:])
    pos = np.empty(src.shape[0], np.int64)
    for ci in range(NCORES):
        for b in range(NBLK):
            for r in (0, 1):
                off = starts[ci, b, r]
                cnt = counts[ci, b, r]
                pos[off:off + cnt] = col0[b, r] * P + np.arange(cnt)

    idx_flat = np.zeros((NCORES, totb * P), np.int16)
    dloc_flat = np.zeros((NCORES, totb * P), np.float32)
    norm_flat = np.zeros((NCORES, totb * P), np.float32)
    core_off = np.concatenate([[0], np.cumsum(counts.sum(axis=(1, 2)))])
    sadj = (src - rng * LO).astype(np.int16)
    for ci in range(NCORES):
        sl = slice(core_off[ci], core_off[ci + 1])
        idx_flat[ci, pos[sl]] = sadj[sl]
        dloc_flat[ci, pos[sl]] = dloc[sl]
        norm_flat[ci, pos[sl]] = norm[sl]

    # dma_gather index layout: [128, n/16] int16, idx i at [i%16, i//16],
    # replicated across the 8 gpsimd cores' 16-partition stripes
    idx_w = idx_flat.reshape(NCORES, totb * P // 16, 16).transpose(0, 2, 1)
    idx_w = np.ascontiguousarray(np.tile(idx_w, (1, 8, 1)))

    def col_major(a):
        return np.ascontiguousarray(
            a.reshape(NCORES, totb, P).transpose(0, 2, 1))

    return {
        "nb": nb, "groups": groups, "col0": col0, "totb": totb,
        "chunks": chunks, "grp_chunks": grp_chunks, "idx_w": idx_w,
        "colblk": colblk,
        "dloc_c": col_major(dloc_flat).astype(ml_dtypes.bfloat16),
        "norm_c": col_major(norm_flat),
    }


def _build(pp):
    import concourse.mybir as mybir
    import concourse.tile as tile
    from concourse import bacc
    from concourse.library_config import mlp

    stage = os.environ.get("GCN_STAGE", "full")  # l1 | nocc | full
    nb, groups, col0, totb, chunks, grp_chunks = (
        pp["nb"], pp["groups"], pp["col0"], pp["totb"], pp["chunks"],
        pp["grp_chunks"])
    colblk = pp["colblk"]

    nc = bacc.Bacc("TRN2", target_bir_lowering=False, debug=False,
                   num_devices=NCORES, num_swdge_queues=NQ,
                   dynamic_dma_scratch_size=16384 * min(NQ, 2))
    f32, bf16 = mybir.dt.float32, mybir.dt.bfloat16

    x_d = nc.dram_tensor("x", [N, D], f32, kind="ExternalInput")
    idx_d = nc.dram_tensor("idxs", [P, totb * 8], mybir.dt.int16,
                           kind="ExternalInput")
    dl_d = nc.dram_tensor("dstloc", [P, totb], bf16, kind="ExternalInput")
    nm_d = nc.dram_tensor("normv", [P, totb], f32, kind="ExternalInput")
    io_d = nc.dram_tensor("iota_in", [P, CHUNK * P], bf16,
                          kind="ExternalInput")
    w1_d = nc.dram_tensor("w1", [D, D], f32, kind="ExternalInput")
    w2_d = nc.dram_tensor("w2", [D, D], f32, kind="ExternalInput")
    wc_d = nc.dram_tensor("wc", [D, NCLS], f32, kind="ExternalInput")
    b1_d = nc.dram_tensor("b1", [D, 1], f32, kind="ExternalInput")
    b2_d = nc.dram_tensor("b2", [D, 1], f32, kind="ExternalInput")
    bc_d = nc.dram_tensor("bc_rep", [P, NCLS], f32, kind="ExternalInput")
    out_d = nc.dram_tensor("out", [NPC, NCLS], f32, kind="ExternalOutput")

    relu = mybir.ActivationFunctionType.Relu
    is_eq = mybir.AluOpType.is_equal
    mult = mybir.AluOpType.mult

    with tile.TileContext(nc) as tc:
        with (
            tc.tile_pool(name="const", bufs=1) as const,
            tc.tile_pool(name="gpool", bufs=2) as gpool,
            tc.tile_pool(name="mpool", bufs=2) as mpool,
            tc.tile_pool(name="epil", bufs=4) as epil,
            tc.tile_pool(name="pacc", bufs=1, space="PSUM") as pacc,
            tc.tile_pool(name="pepi", bufs=3, space="PSUM") as pepi,
        ):
            nc.gpsimd.load_library(mlp)

            idx_sb = const.tile([P, totb * 8], mybir.dt.int16)
            nc.sync.dma_start(idx_sb[:], idx_d[:])
            dl_sb = const.tile([P, totb], bf16)
            nc.sync.dma_start(dl_sb[:], dl_d[:])
            nm_sb = const.tile([P, totb], f32)
            nc.sync.dma_start(nm_sb[:], nm_d[:])
            io_sb = const.tile([P, CHUNK * P], bf16)
            nc.sync.dma_start(io_sb[:], io_d[:])
            w1_sb = const.tile([D, D], f32)
            nc.sync.dma_start(w1_sb[:], w1_d[:])
            w2_sb = const.tile([D, D], f32)
            nc.sync.dma_start(w2_sb[:], w2_d[:])
            wc_sb = const.tile([D, NCLS], f32)
            nc.sync.dma_start(wc_sb[:], wc_d[:])
            b1_sb = const.tile([D, 1], f32)
            nc.sync.dma_start(b1_sb[:], b1_d[:])
            b2_sb = const.tile([D, 1], f32)
            nc.sync.dma_start(b2_sb[:], b2_d[:])
            bc_sb = const.tile([P, NCLS], f32)
            nc.sync.dma_start(bc_sb[:], bc_d[:])

            t2_slice = nc.dram_tensor("t2_slice", [NPC, D], f32)
            t2_full = nc.dram_tensor("t2_full", [N, D], f32,
                                     addr_space="Shared")

            def gather_chunk(tbl_lo, tbl_hi, c0, c1, r):
                """One chunk = one gather tile, split across NQ queues."""
                w = c1 - c0
                gt = gpool.tile([P, CHUNK, D], f32, tag="g")
                tbl = tbl_lo if r == 0 else tbl_hi
                per = math.ceil(w / NQ)
                q = 0
                for s in range(0, w, per):
                    e = min(s + per, w)
                    n_idx = (e - s) * P
                    nc.gpsimd.dma_gather(
                        gt[:, s:e, :], tbl,
                        idx_sb[:, (c0 + s) * 8:(c0 + e) * 8],
                        n_idx, n_idx, D, single_packet=False,
                        queue_num=q % NQ)
                    q += 1
                return gt

            def epilogue(b, aslc, is_last):
                rows = min(P, NPC - b * P)
                if not is_last:
                    ag = epil.tile([D, P], f32, tag="ag")
                    nc.vector.tensor_copy(ag[:], aslc)
                    hp = pepi.tile([D, P], f32, tag="epi")
                    nc.tensor.matmul(hp[:], w1_sb[:], ag[:],
                                     start=True, stop=True)
                    h2 = epil.tile([D, P], f32, tag="h2")
                    nc.scalar.activation(h2[:], hp[:], relu,
                                         bias=b1_sb[:, :1])
                    tp = pepi.tile([P, D], f32, tag="epi")
                    nc.tensor.matmul(tp[:], h2[:], w2_sb[:],
                                     start=True, stop=True)
                    ts = epil.tile([P, D], f32, tag="ts")
                    nc.vector.tensor_copy(ts[:], tp[:])
                    nc.sync.dma_start(
                        t2_slice[b * P:b * P + rows, :], ts[:rows, :])
                    if stage == "l1":
                        nc.sync.dma_start(
                            out_d[b * P:b * P + rows, :], ts[:rows, :NCLS])
                else:
                    h3 = epil.tile([D, P], f32, tag="h2")
                    nc.scalar.activation(h3[:], aslc, relu,
                                         bias=b2_sb[:, :1])
                    op = pepi.tile([P, NCLS], f32, tag="epi")
                    nc.tensor.matmul(op[:], h3[:], wc_sb[:],
                                     start=True, stop=True)
                    os_ = epil.tile([P, NCLS], f32, tag="ts")
                    nc.vector.tensor_add(os_[:], op[:], bc_sb[:])
                    nc.sync.dma_start(
                        out_d[b * P:b * P + rows, :], os_[:rows, :])

            def layer(tbl_lo, tbl_hi, is_last):
                for gi, blocks in enumerate(groups):
                    accs = [pacc.tile([D, P], f32, tag=f"acc{bi}",
                                       name=f"acc{bi}")
                            for bi in range(len(blocks))]
                    done = {b: 0 for b in blocks}
                    nbtot = {b: int(nb[b, 0] + nb[b, 1]) for b in blocks}
                    for cid in grp_chunks[gi]:
                        c0, c1, r = chunks[cid]
                        w = c1 - c0
                        gt = gather_chunk(tbl_lo, tbl_hi, c0, c1, r)
                        mt = mpool.tile([P, CHUNK, P], bf16, tag="m")
                        nc.vector.tensor_tensor(
                            out=mt[:, :w, :],
                            in0=io_sb[:, :w * P].rearrange(
                                "p (b k) -> p b k", k=P),
                            in1=dl_sb[:, c0:c1].to_broadcast([P, w, P]),
                            op=is_eq)
                        mg = mpool.tile([P, CHUNK, D], bf16, tag="mg")
                        nc.vector.tensor_tensor(
                            out=mg[:, :w, :], in0=gt[:, :w, :],
                            in1=nm_sb[:, c0:c1].to_broadcast([P, w, D]),
                            op=mult)
                        for col in range(c0, c1):
                            b = int(colblk[col])
                            bi = blocks.index(b)
                            aslc = accs[bi][:]
                            k = done[b]
                            done[b] = k + 1
                            j = col - c0
                            nc.tensor.matmul(
                                aslc, mg[:, j, :], mt[:, j, :],
                                start=(k == 0), stop=(k == nbtot[b] - 1))
                    # epilogues only after ALL of this group's matmuls, so
                    # DVE reads of acc never share a live PSUM bank with PE
                    # writes (HW read/write hazard corrupts data otherwise).
                    for bi, b in enumerate(blocks):
                        epilogue(b, accs[bi][:], is_last)

            layer(x_d[0:LO, :], x_d[LO:N, :], is_last=False)

            if stage != "l1":
                if stage != "nocc":
                    nc.gpsimd.collective_compute(
                        "AllGather", mybir.AluOpType.bypass,
                        replica_groups=[list(range(NCORES))],
                        ins=[t2_slice[:]], outs=[t2_full[:]])
                layer(t2_full[0:LO, :], t2_full[LO:N, :], is_last=True)

    nc.compile()
    return nc


def _get(edge_index: np.ndarray):
    h = (hashlib.sha1(np.ascontiguousarray(edge_index)).hexdigest()
         + os.environ.get("GCN_STAGE", "full") + str(NQ))
    if h not in _cache:
        pp = _preprocess(np.asarray(edge_index, dtype=np.int64))
        nc = _build(pp)
        _cache.clear()
        _cache[h] = (pp, nc)
    return _cache[h]


def kernel(x, edge_index, W1, b1, W2, b2, Wc, bc):
    import ml_dtypes
    from concourse.bass_utils import run_bass_kernel_spmd

    pp, nc = _get(np.asarray(edge_index))
    x = np.ascontiguousarray(np.asarray(x, np.float32))
    iota = np.tile(np.arange(P, dtype=np.float32),
                   (P, CHUNK)).astype(ml_dtypes.bfloat16)
    common = {
        "x": x,
        "iota_in": iota,
        "w1": np.ascontiguousarray(np.asarray(W1, np.float32)),
        "w2": np.ascontiguousarray(np.asarray(W2, np.float32)),
        "wc": np.ascontiguousarray(np.asarray(Wc, np.float32)),
        "b1": np.asarray(b1, np.float32).reshape(D, 1),
        "b2": np.asarray(b2, np.float32).reshape(D, 1),
        "bc_rep": np.tile(np.asarray(bc, np.float32).reshape(1, NCLS),
                          (P, 1)),
    }
    in_maps = [dict(common,
                    idxs=pp["idx_w"][c],
                    dstloc=pp["dloc_c"][c],
                    normv=pp["norm_c"][c]) for c in range(NCORES)]

    trace = bool(int(os.environ.get("GCN_TRACE", "0")))
    kw = {}
    if trace:
        kw["tmpdir"] = os.environ.get("GCN_TRACE_DIR") or None
        tc_env = os.environ.get("GCN_TRACE_CORES", "0")
        kw["trace_cores"] = [int(c) for c in tc_env.split(",")]
    res = run_bass_kernel_spmd(
        nc, in_maps, core_ids=list(range(NCORES)), trace=trace, **kw)
    if res.exec_time_ns is not None:
        print(f"HW exec time: {res.exec_time_ns} ns")
        if res.instructions_and_trace is not None:
            print(f"trace: {res.instructions_and_trace[1]}")
    out = np.concatenate([res.results[c]["out"] for c in range(NCORES)],
                         axis=0)
    return out.astype(np.float32)


# revision 23
# speedup vs baseline: 2.6620x; 1.1649x over previous
"""2-layer GCN + linear classifier on 8 trn2 NeuronCores.

out = relu(A @ relu(A @ x @ W1 + b1) @ W2 + b2) @ Wc + bc
with A = D^-1/2 (adj + I) D^-1/2, N=50000 nodes, E=800000 edges, d=64.

Strategy (dst-partitioned graph parallel):
 - Host routes edges by dst shard (6250 nodes/core), sorts per 128-node dst
   block, splits by src < 32768 (dma_gather int16 index limit), pads each
   (block, range) group to whole 128-edge batches uniformly across cores
   (same SPMD program everywhere), and precomputes per-edge norm =
   rsqrt(deg[src]) * rsqrt(deg[dst]) plus in-block dst offsets.
 - Device, per gather chunk (<=64 batches): dma_gather split across 4 SWDGE
   queues pulls the source feature rows (x for layer 1, t2 = h2 @ W2 for
   layer 2); one bulk DVE tensor_tensor builds the one-hot selection
   matrices M[e, d] = (dstloc[e] == d) in bf16; one bulk tensor_tensor
   scales messages by norm and casts to bf16. PE contracts msg.T @ M per
   128-edge batch, accumulating aggT [64f, 128n] per dst block in PSUM.
   Layer-1 epilogue per block: aggT -> @W1 -> relu+b1 -> @W2 -> t2 slice
   in DRAM. One AllGather exchanges t2 slices between layers. Layer-2
   epilogue: relu+b2 -> @Wc -> +bc -> out slice.
"""

import hashlib
import math
import os

import numpy as np

N = 50000
E = 800000
D = 64
NCLS = 16
NCORES = 8
NPC = N // NCORES            # 6250 nodes per core
P = 128
NBLK = math.ceil(NPC / P)    # 49 dst blocks per core (last has 106 nodes)
LO = 32768                   # int16 index range split
G = 4                        # dst blocks per PSUM group (1 PSUM bank each)
CHUNK = 48                   # max batches per gather chunk
NQ = int(os.environ.get("GCN_NQ", "4"))      # swdge queues

_cache: dict = {}


def _preprocess(edge_index: np.ndarray):
    """Route/sort/pad edges; build per-core device arrays."""
    import ml_dtypes

    src = np.concatenate([edge_index[0], np.arange(N, dtype=np.int64)])
    dst = np.concatenate([edge_index[1], np.arange(N, dtype=np.int64)])
    deg = np.bincount(dst, minlength=N).astype(np.float64)
    inv = 1.0 / np.sqrt(np.maximum(deg, 1.0))
    inv[deg == 0] = 0.0
    norm = (inv[src] * inv[dst]).astype(np.float32)

    core = dst // NPC
    ldst = dst - core * NPC
    blk = ldst // P
    dloc = (ldst % P).astype(np.float32)
    rng = (src >= LO).astype(np.int64)

    key = (core * NBLK + blk) * 2 + rng
    order = np.argsort(key, kind="stable")
    src, norm, dloc, rng = src[order], norm[order], dloc[order], rng[order]

    counts = np.bincount(key[order], minlength=NCORES * NBLK * 2).reshape(
        NCORES, NBLK, 2)
    nb = np.ceil(counts / P).astype(np.int64).max(axis=0)      # [NBLK, 2]

    groups = [list(range(g * G, min((g + 1) * G, NBLK)))
              for g in range(math.ceil(NBLK / G))]
    col0 = np.zeros((NBLK, 2), np.int64)
    chunks = []          # (c0, c1, range) batch-col spans
    grp_chunks = []      # per group: list of chunk ids
    c = 0
    for blocks in groups:
        ids = []
        for r in (0, 1):
            s = c
            for b in blocks:
                col0[b, r] = c
                c += nb[b, r]
            e = c
            cc = s
            while cc < e:
                ids.append(len(chunks))
                chunks.append((cc, min(cc + CHUNK, e), r))
                cc = min(cc + CHUNK, e)
        grp_chunks.append(ids)
    totb = c

    colblk = np.zeros(totb, np.int64)          # batch col -> dst block
    for b in range(NBLK):
        for r in (0, 1):
            colblk[col0[b, r]:col0[b, r] + nb[b, r]] = b

    # flat position of every edge within its core's [totb*128] stream
    starts = np.zeros((NCORES, NBLK, 2), np.int64)
    flat = counts.reshape(-1)
    np.cumsum(flat[:-1], out=starts.reshape(-1)[1:])
    pos = np.empty(src.shape[0], np.int64)
    for ci in range(NCORES):
        for b in range(NBLK):
            for r in (0, 1):
                off = starts[ci, b, r]
                cnt = counts[ci, b, r]
                pos[off:off + cnt] = col0[b, r] * P + np.arange(cnt)

    idx_flat = np.zeros((NCORES, totb * P), np.int16)
    dloc_flat = np.zeros((NCORES, totb * P), np.float32)
    norm_flat = np.zeros((NCORES, totb * P), np.float32)
    core_off = np.concatenate([[0], np.cumsum(counts.sum(axis=(1, 2)))])
    sadj = (src - rng * LO).astype(np.int16)
    for ci in range(NCORES):
        sl = slice(core_off[ci], core_off[ci + 1])
        idx_flat[ci, pos[sl]] = sadj[sl]
        dloc_flat[ci, pos[sl]] = dloc[sl]
        norm_flat[ci, pos[sl]] = norm[sl]

    # dma_gather index layout: [128, n/16] int16, idx i at [i%16, i//16],
    # replicated across the 8 gpsimd cores' 16-partition stripes
    idx_w = idx_flat.reshape(NCORES, totb * P // 16, 16).transpose(0, 2, 1)
    idx_w = np.ascontiguousarray(np.tile(idx_w, (1, 8, 1)))

    def col_major(a):
        return np.ascontiguousarray(
            a.reshape(NCORES, totb, P).transpose(0, 2, 1))

    return {
        "nb": nb, "groups": groups, "col0": col0, "totb": totb,
        "chunks": chunks, "grp_chunks": grp_chunks, "idx_w": idx_w,
        "colblk": colblk,
        "dloc_c": col_major(dloc_flat).astype(ml_dtypes.bfloat16),
        "norm_c": col_major(norm_flat),
    }


def _build(pp):
    import concourse.mybir as mybir
    import concourse.tile as tile
    from concourse import bacc
    from concourse.library_config import mlp

    stage = os.environ.get("GCN_STAGE", "full")  # l1 | nocc | full
    nb, groups, col0, totb, chunks, grp_chunks = (
        pp["nb"], pp["groups"], pp["col0"], pp["totb"], pp["chunks"],
        pp["grp_chunks"])
    colblk = pp["colblk"]

    nc = bacc.Bacc("TRN2", target_bir_lowering=False, debug=False,
                   num_devices=NCORES, num_swdge_queues=NQ,
                   dynamic_dma_scratch_size=16384 * min(NQ, 2))
    f32, bf16 = mybir.dt.float32, mybir.dt.bfloat16

    x_d = nc.dram_tensor("x", [N, D], f32, kind="ExternalInput")
    idx_d = nc.dram_tensor("idxs", [P, totb * 8], mybir.dt.int16,
                           kind="ExternalInput")
    dl_d = nc.dram_tensor("dstloc", [P, totb], bf16, kind="ExternalInput")
    nm_d = nc.dram_tensor("normv", [P, totb], f32, kind="ExternalInput")
    io_d = nc.dram_tensor("iota_in", [P, CHUNK * P], bf16,
                          kind="ExternalInput")
    w1_d = nc.dram_tensor("w1", [D, D], f32, kind="ExternalInput")
    w2_d = nc.dram_tensor("w2", [D, D], f32, kind="ExternalInput")
    wc_d = nc.dram_tensor("wc", [D, NCLS], f32, kind="ExternalInput")
    b1_d = nc.dram_tensor("b1", [D, 1], f32, kind="ExternalInput")
    b2_d = nc.dram_tensor("b2", [D, 1], f32, kind="ExternalInput")
    bc_d = nc.dram_tensor("bc_rep", [P, NCLS], f32, kind="ExternalInput")
    out_d = nc.dram_tensor("out", [NPC, NCLS], f32, kind="ExternalOutput")

    relu = mybir.ActivationFunctionType.Relu
    is_eq = mybir.AluOpType.is_equal
    mult = mybir.AluOpType.mult

    with tile.TileContext(nc) as tc:
        with (
            tc.tile_pool(name="const", bufs=1) as const,
            tc.tile_pool(name="gpool", bufs=3) as gpool,
            tc.tile_pool(name="mpool", bufs=3) as mpool,
            tc.tile_pool(name="epil", bufs=4) as epil,
            tc.tile_pool(name="pacc", bufs=1, space="PSUM") as pacc,
            tc.tile_pool(name="pepi", bufs=3, space="PSUM") as pepi,
        ):
            nc.gpsimd.load_library(mlp)

            idx_sb = const.tile([P, totb * 8], mybir.dt.int16)
            nc.sync.dma_start(idx_sb[:], idx_d[:])
            dl_sb = const.tile([P, totb], bf16)
            nc.sync.dma_start(dl_sb[:], dl_d[:])
            nm_sb = const.tile([P, totb], f32)
            nc.sync.dma_start(nm_sb[:], nm_d[:])
            io_sb = const.tile([P, CHUNK * P], bf16)
            nc.sync.dma_start(io_sb[:], io_d[:])
            w1_sb = const.tile([D, D], f32)
            nc.sync.dma_start(w1_sb[:], w1_d[:])
            w2_sb = const.tile([D, D], f32)
            nc.sync.dma_start(w2_sb[:], w2_d[:])
            wc_sb = const.tile([D, NCLS], f32)
            nc.sync.dma_start(wc_sb[:], wc_d[:])
            b1_sb = const.tile([D, 1], f32)
            nc.sync.dma_start(b1_sb[:], b1_d[:])
            b2_sb = const.tile([D, 1], f32)
            nc.sync.dma_start(b2_sb[:], b2_d[:])
            bc_sb = const.tile([P, NCLS], f32)
            nc.sync.dma_start(bc_sb[:], bc_d[:])

            t2_slice = nc.dram_tensor("t2_slice", [NPC, D], f32)
            t2_full = nc.dram_tensor("t2_full", [N, D], f32,
                                     addr_space="Shared")

            def gather_chunk(tbl_lo, tbl_hi, c0, c1, r):
                """One chunk = one gather tile, split across NQ queues."""
                w = c1 - c0
                gt = gpool.tile([P, CHUNK, D], f32, tag="g")
                tbl = tbl_lo if r == 0 else tbl_hi
                per = math.ceil(w / NQ)
                q = 0
                for s in range(0, w, per):
                    e = min(s + per, w)
                    n_idx = (e - s) * P
                    nc.gpsimd.dma_gather(
                        gt[:, s:e, :], tbl,
                        idx_sb[:, (c0 + s) * 8:(c0 + e) * 8],
                        n_idx, n_idx, D, single_packet=False,
                        queue_num=q % NQ)
                    q += 1
                return gt

            def epilogue(b, aslc, is_last):
                rows = min(P, NPC - b * P)
                if not is_last:
                    ag = epil.tile([D, P], f32, tag="ag")
                    nc.vector.tensor_copy(ag[:], aslc)
                    hp = pepi.tile([D, P], f32, tag="epi")
                    nc.tensor.matmul(hp[:], w1_sb[:], ag[:],
                                     start=True, stop=True)
                    h2 = epil.tile([D, P], f32, tag="h2")
                    nc.scalar.activation(h2[:], hp[:], relu,
                                         bias=b1_sb[:, :1])
                    tp = pepi.tile([P, D], f32, tag="epi")
                    nc.tensor.matmul(tp[:], h2[:], w2_sb[:],
                                     start=True, stop=True)
                    ts = epil.tile([P, D], f32, tag="ts")
                    nc.vector.tensor_copy(ts[:], tp[:])
                    nc.sync.dma_start(
                        t2_slice[b * P:b * P + rows, :], ts[:rows, :])
                    if stage == "l1":
                        nc.sync.dma_start(
                            out_d[b * P:b * P + rows, :], ts[:rows, :NCLS])
                else:
                    h3 = epil.tile([D, P], f32, tag="h2")
                    nc.scalar.activation(h3[:], aslc, relu,
                                         bias=b2_sb[:, :1])
                    op = pepi.tile([P, NCLS], f32, tag="epi")
                    nc.tensor.matmul(op[:], h3[:], wc_sb[:],
                                     start=True, stop=True)
                    os_ = epil.tile([P, NCLS], f32, tag="ts")
                    nc.vector.tensor_add(os_[:], op[:], bc_sb[:])
                    nc.sync.dma_start(
                        out_d[b * P:b * P + rows, :], os_[:rows, :])

            def layer(tbl_lo, tbl_hi, is_last):
                for gi, blocks in enumerate(groups):
                    accs = [pacc.tile([D, P], f32, tag=f"acc{bi}",
                                       name=f"acc{bi}")
                            for bi in range(len(blocks))]
                    done = {b: 0 for b in blocks}
                    nbtot = {b: int(nb[b, 0] + nb[b, 1]) for b in blocks}
                    for cid in grp_chunks[gi]:
                        c0, c1, r = chunks[cid]
                        w = c1 - c0
                        gt = gather_chunk(tbl_lo, tbl_hi, c0, c1, r)
                        mt = mpool.tile([P, CHUNK, P], bf16, tag="m")
                        nc.vector.tensor_tensor(
                            out=mt[:, :w, :],
                            in0=io_sb[:, :w * P].rearrange(
                                "p (b k) -> p b k", k=P),
                            in1=dl_sb[:, c0:c1].to_broadcast([P, w, P]),
                            op=is_eq)
                        mg = mpool.tile([P, CHUNK, D], bf16, tag="mg")
                        nc.vector.tensor_tensor(
                            out=mg[:, :w, :], in0=gt[:, :w, :],
                            in1=nm_sb[:, c0:c1].to_broadcast([P, w, D]),
                            op=mult)
                        for col in range(c0, c1):
                            b = int(colblk[col])
                            bi = blocks.index(b)
                            aslc = accs[bi][:]
                            k = done[b]
                            done[b] = k + 1
                            j = col - c0
                            nc.tensor.matmul(
                                aslc, mg[:, j, :], mt[:, j, :],
                                start=(k == 0), stop=(k == nbtot[b] - 1))
                    # epilogues only after ALL of this group's matmuls, so
                    # DVE reads of acc never share a live PSUM bank with PE
                    # writes (HW read/write hazard corrupts data otherwise).
                    for bi, b in enumerate(blocks):
                        epilogue(b, accs[bi][:], is_last)

            layer(x_d[0:LO, :], x_d[LO:N, :], is_last=False)

            if stage != "l1":
                if stage != "nocc":
                    nc.gpsimd.collective_compute(
                        "AllGather", mybir.AluOpType.bypass,
                        replica_groups=[list(range(NCORES))],
                        ins=[t2_slice[:]], outs=[t2_full[:]])
                layer(t2_full[0:LO, :], t2_full[LO:N, :], is_last=True)

    nc.compile()
    return nc


def _get(edge_index: np.ndarray):
    h = (hashlib.sha1(np.ascontiguousarray(edge_index)).hexdigest()
         + os.environ.get("GCN_STAGE", "full") + str(NQ))
    if h not in _cache:
        pp = _preprocess(np.asarray(edge_index, dtype=np.int64))
        nc = _build(pp)
        _cache.clear()
        _cache[h] = (pp, nc)
    return _cache[h]


def kernel(x, edge_index, W1, b1, W2, b2, Wc, bc):
    import ml_dtypes
    from concourse.bass_utils import run_bass_kernel_spmd

    pp, nc = _get(np.asarray(edge_index))
    x = np.ascontiguousarray(np.asarray(x, np.float32))
    iota = np.tile(np.arange(P, dtype=np.float32),
                   (P, CHUNK)).astype(ml_dtypes.bfloat16)
    common = {
        "x": x,
        "iota_in": iota,
        "w1": np.ascontiguousarray(np.asarray(W1, np.float32)),
        "w2": np.ascontiguousarray(np.asarray(W2, np.float32)),
        "wc": np.ascontiguousarray(np.asarray(Wc, np.float32)),
        "b1": np.asarray(b1, np.float32).reshape(D, 1),
        "b2": np.asarray(b2, np.float32).reshape(D, 1),
        "bc_rep": np.tile(np.asarray(bc, np.float32).reshape(1, NCLS),
                          (P, 1)),
    }
    in_maps = [dict(common,
                    idxs=pp["idx_w"][c],
                    dstloc=pp["dloc_c"][c],
                    normv=pp["norm_c"][c]) for c in range(NCORES)]

    trace = bool(int(os.environ.get("GCN_TRACE", "0")))
    kw = {}
    if trace:
        kw["tmpdir"] = os.environ.get("GCN_TRACE_DIR") or None
        tc_env = os.environ.get("GCN_TRACE_CORES", "0")
        kw["trace_cores"] = [int(c) for c in tc_env.split(",")]
    res = run_bass_kernel_spmd(
        nc, in_maps, core_ids=list(range(NCORES)), trace=trace, **kw)
    if res.exec_time_ns is not None:
        print(f"HW exec time: {res.exec_time_ns} ns")
        if res.instructions_and_trace is not None:
            print(f"trace: {res.instructions_and_trace[1]}")
    out = np.concatenate([res.results[c]["out"] for c in range(NCORES)],
                         axis=0)
    return out.astype(np.float32)


# revision 29
# speedup vs baseline: 2.8987x; 1.0889x over previous
"""2-layer GCN + linear classifier on 8 trn2 NeuronCores.

out = relu(A @ relu(A @ x @ W1 + b1) @ W2 + b2) @ Wc + bc
with A = D^-1/2 (adj + I) D^-1/2, N=50000 nodes, E=800000 edges, d=64.

Strategy (dst-partitioned graph parallel):
 - Host routes edges by dst shard (6250 nodes/core), sorts per 128-node dst
   block, pads each block's edge list to whole 128-edge batches uniformly
   across cores (same SPMD program everywhere), and precomputes per-edge
   norm = rsqrt(deg[src]) * rsqrt(deg[dst]) plus in-block dst offsets.
 - Feature tables are stored as [rows/2, 128] (two nodes per 512B row) so a
   single dma_gather descriptor fetches node pair src//2 — this halves the
   SWDGE descriptor count (the serial bottleneck) and keeps indices within
   dma_gather's int16 range with no table split. Per-edge parity masks
   (norm_even/norm_odd) zero out the wrong half when messages are scaled.
 - Device, per gather chunk (<=32 batches): dma_gather split across 4 SWDGE
   queues; one bulk DVE tensor_tensor builds one-hot selection matrices
   M[e, d] = (dstloc[e] == d) in bf16; two bulk tensor_tensors scale the
   left/right row halves by norm_even/norm_odd into a stacked message tile
   mg[e, 0:64|64:128] (bf16). One PE matmul per 128-edge batch computes
   mg.T @ M into a per-block accumulator [128(2x64f), 128n]; the two
   64-feature halves sum to the true aggregate. Layer-1 epilogue folds that
   sum into a stacked-[W1;W1] matmul -> relu+b1 -> @W2 -> t2 slice. One
   AllGather exchanges t2 slices. Layer-2 epilogue: halves-add -> relu+b2
   -> @Wc -> +bc -> out slice.
"""

import hashlib
import math
import os

import numpy as np

N = 50000
E = 800000
D = 64
NCLS = 16
NCORES = 8
NPC = N // NCORES            # 6250 nodes per core
P = 128
NBLK = math.ceil(NPC / P)    # 49 dst blocks per core (last has 106 nodes)
G = 4                        # dst blocks per PSUM group (1 PSUM bank each)
CHUNK = 32                   # max batches per gather chunk
NQ = int(os.environ.get("GCN_NQ", "4"))      # swdge queues
NROW2 = N // 2               # paired-node table rows
TPAD = 25024                 # padded x2 table rows

_cache: dict = {}


def _preprocess(edge_index: np.ndarray):
    """Route/sort/pad edges; build per-core device arrays."""
    import ml_dtypes

    src = np.concatenate([edge_index[0], np.arange(N, dtype=np.int64)])
    dst = np.concatenate([edge_index[1], np.arange(N, dtype=np.int64)])
    deg = np.bincount(dst, minlength=N).astype(np.float64)
    inv = 1.0 / np.sqrt(np.maximum(deg, 1.0))
    inv[deg == 0] = 0.0
    norm = (inv[src] * inv[dst]).astype(np.float32)

    core = dst // NPC
    ldst = dst - core * NPC
    blk = ldst // P
    dloc = (ldst % P).astype(np.float32)

    key = core * NBLK + blk
    order = np.argsort(key, kind="stable")
    src, norm, dloc = src[order], norm[order], dloc[order]

    counts = np.bincount(key[order], minlength=NCORES * NBLK).reshape(
        NCORES, NBLK)
    nb = np.ceil(counts / P).astype(np.int64).max(axis=0)      # [NBLK]

    groups = [list(range(g * G, min((g + 1) * G, NBLK)))
              for g in range(math.ceil(NBLK / G))]
    col0 = np.zeros(NBLK, np.int64)
    np.cumsum(nb[:-1], out=col0[1:])
    totb = int(nb.sum())

    chunks = []          # (c0, c1) batch-col spans
    grp_chunks = []
    for blocks in groups:
        s = int(col0[blocks[0]])
        e = int(col0[blocks[-1]] + nb[blocks[-1]])
        ids = []
        c = s
        while c < e:
            ids.append(len(chunks))
            chunks.append((c, min(c + CHUNK, e)))
            c = min(c + CHUNK, e)
        grp_chunks.append(ids)

    colblk = np.zeros(totb, np.int64)
    for b in range(NBLK):
        colblk[col0[b]:col0[b] + nb[b]] = b

    starts = np.zeros((NCORES, NBLK), np.int64)
    flat = counts.reshape(-1)
    np.cumsum(flat[:-1], out=starts.reshape(-1)[1:])
    pos = np.empty(src.shape[0], np.int64)
    for ci in range(NCORES):
        for b in range(NBLK):
            off = starts[ci, b]
            cnt = counts[ci, b]
            pos[off:off + cnt] = col0[b] * P + np.arange(cnt)

    idx_flat = np.zeros((NCORES, totb * P), np.int16)
    dloc_flat = np.zeros((NCORES, totb * P), np.float32)
    nme_flat = np.zeros((NCORES, totb * P), np.float32)
    nmo_flat = np.zeros((NCORES, totb * P), np.float32)
    core_off = np.concatenate([[0], np.cumsum(counts.sum(axis=1))])
    half = (src // 2).astype(np.int16)
    par = (src & 1).astype(np.float32)
    for ci in range(NCORES):
        sl = slice(core_off[ci], core_off[ci + 1])
        idx_flat[ci, pos[sl]] = half[sl]
        dloc_flat[ci, pos[sl]] = dloc[sl]
        nme_flat[ci, pos[sl]] = norm[sl] * (1.0 - par[sl])
        nmo_flat[ci, pos[sl]] = norm[sl] * par[sl]

    idx_w = idx_flat.reshape(NCORES, totb * P // 16, 16).transpose(0, 2, 1)
    idx_w = np.ascontiguousarray(np.tile(idx_w, (1, 8, 1)))

    def col_major(a):
        return np.ascontiguousarray(
            a.reshape(NCORES, totb, P).transpose(0, 2, 1))

    return {
        "nb": nb, "groups": groups, "col0": col0, "totb": totb,
        "chunks": chunks, "grp_chunks": grp_chunks, "idx_w": idx_w,
        "colblk": colblk,
        "dloc_c": col_major(dloc_flat).astype(ml_dtypes.bfloat16),
        "nme_c": col_major(nme_flat), "nmo_c": col_major(nmo_flat),
    }


def _build(pp):
    import concourse.mybir as mybir
    import concourse.tile as tile
    from concourse import bacc
    from concourse.library_config import mlp

    stage = os.environ.get("GCN_STAGE", "full")  # l1 | nocc | full
    nb, groups, col0, totb, chunks, grp_chunks = (
        pp["nb"], pp["groups"], pp["col0"], pp["totb"], pp["chunks"],
        pp["grp_chunks"])
    colblk = pp["colblk"]

    nc = bacc.Bacc("TRN2", target_bir_lowering=False, debug=False,
                   num_devices=NCORES, num_swdge_queues=NQ,
                   dynamic_dma_scratch_size=32768)
    f32, bf16 = mybir.dt.float32, mybir.dt.bfloat16

    x2_d = nc.dram_tensor("x2", [TPAD, 2 * D], f32, kind="ExternalInput")
    idx_d = nc.dram_tensor("idxs", [P, totb * 8], mybir.dt.int16,
                           kind="ExternalInput")
    dl_d = nc.dram_tensor("dstloc", [P, totb], bf16, kind="ExternalInput")
    nme_d = nc.dram_tensor("nme", [P, totb], f32, kind="ExternalInput")
    nmo_d = nc.dram_tensor("nmo", [P, totb], f32, kind="ExternalInput")
    io_d = nc.dram_tensor("iota_in", [P, CHUNK * P], bf16,
                          kind="ExternalInput")
    w12_d = nc.dram_tensor("w12", [2 * D, D], f32, kind="ExternalInput")
    w2_d = nc.dram_tensor("w2", [D, D], f32, kind="ExternalInput")
    wc_d = nc.dram_tensor("wc", [D, NCLS], f32, kind="ExternalInput")
    b1_d = nc.dram_tensor("b1", [D, 1], f32, kind="ExternalInput")
    b2_d = nc.dram_tensor("b2", [D, 1], f32, kind="ExternalInput")
    bc_d = nc.dram_tensor("bc_rep", [P, NCLS], f32, kind="ExternalInput")
    i2_d = nc.dram_tensor("i2", [2 * D, D], f32, kind="ExternalInput")
    out_d = nc.dram_tensor("out", [NPC, NCLS], f32, kind="ExternalOutput")

    relu = mybir.ActivationFunctionType.Relu
    is_eq = mybir.AluOpType.is_equal
    mult = mybir.AluOpType.mult

    with tile.TileContext(nc) as tc:
        with (
            tc.tile_pool(name="const", bufs=1) as const,
            tc.tile_pool(name="gpool", bufs=3) as gpool,
            tc.tile_pool(name="mpool", bufs=3) as mpool,
            tc.tile_pool(name="epil", bufs=4) as epil,
            tc.tile_pool(name="pacc", bufs=1, space="PSUM") as pacc,
            tc.tile_pool(name="pepi", bufs=3, space="PSUM") as pepi,
        ):
            nc.gpsimd.load_library(mlp)

            idx_sb = const.tile([P, totb * 8], mybir.dt.int16)
            nc.sync.dma_start(idx_sb[:], idx_d[:])
            dl_sb = const.tile([P, totb], bf16)
            nc.sync.dma_start(dl_sb[:], dl_d[:])
            nme_sb = const.tile([P, totb], f32)
            nc.sync.dma_start(nme_sb[:], nme_d[:])
            nmo_sb = const.tile([P, totb], f32)
            nc.sync.dma_start(nmo_sb[:], nmo_d[:])
            io_sb = const.tile([P, CHUNK * P], bf16)
            nc.sync.dma_start(io_sb[:], io_d[:])
            w12_sb = const.tile([2 * D, D], f32)
            nc.sync.dma_start(w12_sb[:], w12_d[:])
            w2_sb = const.tile([D, D], f32)
            nc.sync.dma_start(w2_sb[:], w2_d[:])
            wc_sb = const.tile([D, NCLS], f32)
            nc.sync.dma_start(wc_sb[:], wc_d[:])
            b1_sb = const.tile([D, 1], f32)
            nc.sync.dma_start(b1_sb[:], b1_d[:])
            b2_sb = const.tile([D, 1], f32)
            nc.sync.dma_start(b2_sb[:], b2_d[:])
            bc_sb = const.tile([P, NCLS], f32)
            nc.sync.dma_start(bc_sb[:], bc_d[:])
            i2_sb = const.tile([2 * D, D], f32)
            nc.sync.dma_start(i2_sb[:], i2_d[:])

            t2_slice = nc.dram_tensor("t2_slice", [NPC, D], f32)
            t2_full = nc.dram_tensor("t2_full", [NROW2, 2 * D], f32,
                                     addr_space="Shared")

            def gather_chunk(tbl, c0, c1):
                w = c1 - c0
                gt = gpool.tile([P, CHUNK, 2 * D], f32, tag="g")
                per = math.ceil(w / NQ)
                q = 0
                for s in range(0, w, per):
                    e = min(s + per, w)
                    n_idx = (e - s) * P
                    nc.gpsimd.dma_gather(
                        gt[:, s:e, :], tbl,
                        idx_sb[:, (c0 + s) * 8:(c0 + e) * 8],
                        n_idx, n_idx, 2 * D, single_packet=False,
                        queue_num=q % NQ)
                    q += 1
                return gt

            def epilogue(b, acc, is_last):
                rows = min(P, NPC - b * P)
                ag = epil.tile([P, P], f32, tag="ag")
                nc.vector.tensor_copy(ag[:], acc)
                if not is_last:
                    hp = pepi.tile([D, P], f32, tag="epi")
                    nc.tensor.matmul(hp[:], w12_sb[:], ag[:],
                                     start=True, stop=True)
                    h2 = epil.tile([D, P], f32, tag="h2")
                    nc.scalar.activation(h2[:], hp[:], relu,
                                         bias=b1_sb[:, :1])
                    tp = pepi.tile([P, D], f32, tag="epi")
                    nc.tensor.matmul(tp[:], h2[:], w2_sb[:],
                                     start=True, stop=True)
                    ts = epil.tile([P, D], f32, tag="ts")
                    nc.vector.tensor_copy(ts[:], tp[:])
                    nc.sync.dma_start(
                        t2_slice[b * P:b * P + rows, :], ts[:rows, :])
                    if stage == "l1":
                        nc.sync.dma_start(
                            out_d[b * P:b * P + rows, :], ts[:rows, :NCLS])
                else:
                    hs = pepi.tile([D, P], f32, tag="epi")
                    nc.tensor.matmul(hs[:], i2_sb[:], ag[:],
                                     start=True, stop=True)
                    h3 = epil.tile([D, P], f32, tag="h2")
                    nc.scalar.activation(h3[:], hs[:], relu,
                                         bias=b2_sb[:, :1])
                    op = pepi.tile([P, NCLS], f32, tag="epi")
                    nc.tensor.matmul(op[:], h3[:], wc_sb[:],
                                     start=True, stop=True)
                    os_ = epil.tile([P, NCLS], f32, tag="ts")
                    nc.vector.tensor_add(os_[:], op[:], bc_sb[:])
                    nc.sync.dma_start(
                        out_d[b * P:b * P + rows, :], os_[:rows, :])

            def layer(tbl, is_last):
                for gi, blocks in enumerate(groups):
                    accs = [pacc.tile([P, P], f32, tag=f"acc{bi}",
                                      name=f"acc{bi}")
                            for bi in range(len(blocks))]
                    done = {b: 0 for b in blocks}
                    for cid in grp_chunks[gi]:
                        c0, c1 = chunks[cid]
                        w = c1 - c0
                        gt = gather_chunk(tbl, c0, c1)
                        mt = mpool.tile([P, CHUNK, P], bf16, tag="m")
                        nc.vector.tensor_tensor(
                            out=mt[:, :w, :],
                            in0=io_sb[:, :w * P].rearrange(
                                "p (b k) -> p b k", k=P),
                            in1=dl_sb[:, c0:c1].to_broadcast([P, w, P]),
                            op=is_eq)
                        mg = mpool.tile([P, CHUNK, 2 * D], bf16, tag="mg")
                        nc.vector.tensor_tensor(
                            out=mg[:, :w, :D], in0=gt[:, :w, :D],
                            in1=nme_sb[:, c0:c1].to_broadcast([P, w, D]),
                            op=mult)
                        nc.vector.tensor_tensor(
                            out=mg[:, :w, D:], in0=gt[:, :w, D:],
                            in1=nmo_sb[:, c0:c1].to_broadcast([P, w, D]),
                            op=mult)
                        for col in range(c0, c1):
                            b = int(colblk[col])
                            bi = blocks.index(b)
                            k = done[b]
                            done[b] = k + 1
                            j = col - c0
                            nc.tensor.matmul(
                                accs[bi][:], mg[:, j, :], mt[:, j, :],
                                start=(k == 0), stop=(k == int(nb[b]) - 1))
                    for bi, b in enumerate(blocks):
                        epilogue(b, accs[bi][:], is_last)

            layer(x2_d[:], is_last=False)

            if stage != "l1":
                if stage != "nocc":
                    nc.gpsimd.collective_compute(
                        "AllGather", mybir.AluOpType.bypass,
                        replica_groups=[list(range(NCORES))],
                        ins=[t2_slice[:]], outs=[t2_full[:]])
                layer(t2_full[:], is_last=True)

    nc.compile()
    return nc


def _get(edge_index: np.ndarray):
    h = (hashlib.sha1(np.ascontiguousarray(edge_index)).hexdigest()
         + os.environ.get("GCN_STAGE", "full") + str(NQ))
    if h not in _cache:
        pp = _preprocess(np.asarray(edge_index, dtype=np.int64))
        nc = _build(pp)
        _cache.clear()
        _cache[h] = (pp, nc)
    return _cache[h]


def kernel(x, edge_index, W1, b1, W2, b2, Wc, bc):
    import ml_dtypes
    from concourse.bass_utils import run_bass_kernel_spmd

    pp, nc = _get(np.asarray(edge_index))
    x = np.asarray(x, np.float32)
    x2 = np.zeros((TPAD, 2 * D), np.float32)
    x2[:NROW2] = x.reshape(NROW2, 2 * D)
    W1 = np.asarray(W1, np.float32)
    iota = np.tile(np.arange(P, dtype=np.float32),
                   (P, CHUNK)).astype(ml_dtypes.bfloat16)
    common = {
        "x2": x2,
        "iota_in": iota,
        "w12": np.ascontiguousarray(np.vstack([W1, W1])),
        "w2": np.ascontiguousarray(np.asarray(W2, np.float32)),
        "wc": np.ascontiguousarray(np.asarray(Wc, np.float32)),
        "b1": np.asarray(b1, np.float32).reshape(D, 1),
        "b2": np.asarray(b2, np.float32).reshape(D, 1),
        "bc_rep": np.tile(np.asarray(bc, np.float32).reshape(1, NCLS),
                          (P, 1)),
        "i2": np.ascontiguousarray(
            np.vstack([np.eye(D, dtype=np.float32)] * 2)),
    }
    in_maps = [dict(common,
                    idxs=pp["idx_w"][c],
                    dstloc=pp["dloc_c"][c],
                    nme=pp["nme_c"][c],
                    nmo=pp["nmo_c"][c]) for c in range(NCORES)]

    trace = bool(int(os.environ.get("GCN_TRACE", "0")))
    kw = {}
    if trace:
        kw["tmpdir"] = os.environ.get("GCN_TRACE_DIR") or None
        tc_env = os.environ.get("GCN_TRACE_CORES", "0")
        kw["trace_cores"] = [int(c) for c in tc_env.split(",")]
    res = run_bass_kernel_spmd(
        nc, in_maps, core_ids=list(range(NCORES)), trace=trace, **kw)
    if res.exec_time_ns is not None:
        print(f"HW exec time: {res.exec_time_ns} ns")
        if res.instructions_and_trace is not None:
            print(f"trace: {res.instructions_and_trace[1]}")
    out = np.concatenate([res.results[c]["out"] for c in range(NCORES)],
                         axis=0)
    return out.astype(np.float32)
